# revision 1
# baseline (speedup 1.0000x reference)
"""Trainium2 Bass kernel for nn_MPDWConv (B=8, E=256, H=W=128).

Sharding: data-parallel over batch — each of the 8 NeuronCores processes one
full image.

Per-core algorithm (channel-major layout [c, h, w], guard-padded so every
depthwise-conv tap is a pure free-dim offset):
  1. x (bf16, host-cast) DMA'd into zero-guarded SBUF tiles [128, 130, 130]
     (pad 1), two 128-channel blocks.
  2. Stage-1 3x3 depthwise conv = 9 PSUM-accumulated diagonal matmuls per
     4-row window (lhsT = host-prebuilt diag(w0 tap) bf16 matrices).
     ScalarE copies PSUM->SBUF fusing per-channel bias b0 + bf16 cast into
     x0 tiles [128, 140, 140] (pad 6).
  3. Branch convs on x0: ch 64..127 -> 3x3 dil3 (9 taps, zero-padded diag),
     ch 128..255 -> 5x5 dil3 with shared w2 (25 taps, diag packed twice).
     VectorE copies PSUM->SBUF xc tiles fusing biases.
  4. 1x1 pointwise conv = dense GEMM (2 K-chunks x 2 out-blocks, bf16) with
     b_pw added via a K=1 ones-row matmul; output DMA'd straight from PSUM
     to HBM in f32.
"""

import numpy as np

B, E, H, W = 8, 256, 128, 128
T9 = [(a, b) for a in range(3) for b in range(3)]
T25 = [(a, b) for a in range(5) for b in range(5)]

# Engine split for the depthwise work (tuned via TimelineSim):
# stage-1 window-blocks (64 total) and branch windows (32 total) can run as
# diag-matmuls on PE ('p') or bf16 MAC chains on DVE ('v') / GpSimd ('g').
import os as _os
S1_PATTERN = list(_os.environ.get("S1_PATTERN", "vpvvvvvvpvvvvvvv"))
BR_PATTERN = list(_os.environ.get("BR_PATTERN", "p"))

_CACHE = {}


def _split_excess_waits(nc, mybir):
    """Walrus legalization: TRN2 instructions encode at most ONE sync wait
    (two for EventSemaphore). Tile attaches multi-wait sync_info; move the
    excess onto same-engine NoOp prefixes."""
    n_created = 0
    for fn in nc.m.functions:
        for blk in fn.blocks:
            insts = list(blk.instructions)
            out = []
            changed = False
            for inst in insts:
                si = getattr(inst, "sync_info", None)
                cap = 2 if isinstance(inst, mybir.InstEventSemaphore) else 1
                if si is not None and si.on_wait is not None \
                        and len(si.on_wait) > cap:
                    waits = list(si.on_wait)
                    extra, keep = waits[:-cap], waits[-cap:]
                    for w in extra:
                        n_created += 1
                        nop = mybir.InstNoOp(
                            name=f"I-waitsplit-{n_created}",
                            engine=inst.engine)
                        nop.sync_info = mybir.SyncInfo(
                            on_wait=[w], on_update=[])
                        out.append(nop)
                    inst.sync_info = mybir.SyncInfo(
                        on_wait=keep, on_update=list(si.on_update))
                    changed = True
                out.append(inst)
            if changed:
                blk.instructions = out
    return n_created


def _build_nc():
    import concourse.bass as bass
    import concourse.mybir as mybir
    from concourse import tile

    bf16 = mybir.dt.bfloat16
    f32 = mybir.dt.float32

    nc = bass.Bass(trn_type="TRN2")

    # ---- DRAM parameters ----
    xb = nc.dram_tensor("xb", [2, 128, H, W], bf16, kind="ExternalInput")
    d0 = nc.dram_tensor("d0", [2, 128, 9 * 128], bf16, kind="ExternalInput")
    d1 = nc.dram_tensor("d1", [128, 9 * 128], bf16, kind="ExternalInput")
    d2 = nc.dram_tensor("d2", [128, 25 * 128], bf16, kind="ExternalInput")
    wpw = nc.dram_tensor("wpw", [2, 128, 256], bf16, kind="ExternalInput")
    bpw = nc.dram_tensor("bpw", [2, 128, 1], f32, kind="ExternalInput")
    b0v = nc.dram_tensor("b0v", [2, 128, 1], f32, kind="ExternalInput")
    b1v = nc.dram_tensor("b1v", [128, 1], f32, kind="ExternalInput")
    b2v = nc.dram_tensor("b2v", [128, 1], f32, kind="ExternalInput")
    k0s = nc.dram_tensor("k0s", [2, 128, 9], f32, kind="ExternalInput")
    k1s = nc.dram_tensor("k1s", [128, 9], f32, kind="ExternalInput")
    k2s = nc.dram_tensor("k2s", [128, 25], f32, kind="ExternalInput")
    y = nc.dram_tensor("y", [E, H, W], f32, kind="ExternalOutput")

    xb_ap, y_ap = xb.ap(), y.ap()

    with tile.TileContext(nc) as tc:
        with (
            tc.tile_pool(name="const", bufs=1) as cpool,
            tc.tile_pool(name="xpad", bufs=1) as xpool,
            tc.tile_pool(name="x0pad", bufs=1) as x0pool,
            tc.tile_pool(name="xc", bufs=2) as xcpool,
            tc.tile_pool(name="ystage", bufs=2) as ystage,
            tc.tile_pool(name="ps_s1", bufs=2, space="PSUM") as ps1pool,
            tc.tile_pool(name="ps_br", bufs=2, space="PSUM") as psbrpool,
            tc.tile_pool(name="ps_pw", bufs=2, space="PSUM") as pspwpool,
        ):
            # ---- constants into SBUF ----
            d0t = []
            for blk in range(2):
                t = cpool.tile([128, 9 * 128], bf16, tag=f"d0_{blk}")
                nc.gpsimd.dma_start(out=t[:], in_=d0.ap()[blk])
                d0t.append(t)
            d1t = cpool.tile([128, 9 * 128], bf16, tag="d1")
            nc.gpsimd.dma_start(out=d1t[:], in_=d1.ap())
            d2t = cpool.tile([128, 25 * 128], bf16, tag="d2")
            nc.gpsimd.dma_start(out=d2t[:], in_=d2.ap())
            wpwt = []
            for k in range(2):
                t = cpool.tile([128, 256], bf16, tag=f"wpw_{k}")
                nc.gpsimd.dma_start(out=t[:], in_=wpw.ap()[k])
                wpwt.append(t)
            bpwt = []
            for ob in range(2):
                t = cpool.tile([128, 1], f32, tag=f"bpw_{ob}")
                nc.gpsimd.dma_start(out=t[:], in_=bpw.ap()[ob])
                bpwt.append(t)
            b0t = []
            for blk in range(2):
                t = cpool.tile([128, 1], f32, tag=f"b0_{blk}")
                nc.gpsimd.dma_start(out=t[:], in_=b0v.ap()[blk])
                b0t.append(t)
            b1t = cpool.tile([128, 1], f32, tag="b1")
            nc.gpsimd.dma_start(out=b1t[:], in_=b1v.ap())
            b2t = cpool.tile([128, 1], f32, tag="b2")
            nc.gpsimd.dma_start(out=b2t[:], in_=b2v.ap())
            k0t = []
            for blk in range(2):
                t = cpool.tile([128, 9], f32, tag=f"k0_{blk}")
                nc.gpsimd.dma_start(out=t[:], in_=k0s.ap()[blk])
                k0t.append(t)
            k1t = cpool.tile([128, 9], f32, tag="k1")
            nc.gpsimd.dma_start(out=k1t[:], in_=k1s.ap())
            k2t = cpool.tile([128, 25], f32, tag="k2")
            nc.gpsimd.dma_start(out=k2t[:], in_=k2s.ap())

            # Pre-touch bias tiles on their consumer engines so steady-state
            # ops don't each carry a DMA-lane sync wait (walrus allows at
            # most 2 sync waits per instruction).
            scrA = cpool.tile([128, 1], f32, tag="scrA")
            scrV = cpool.tile([128, 1], f32, tag="scrV")
            nc.scalar.copy(scrA[:], b0t[0][:])
            nc.scalar.copy(scrA[:], b0t[1][:])
            nc.scalar.copy(scrA[:], bpwt[1][:])
            nc.vector.tensor_copy(scrV[:], b1t[:])
            nc.vector.tensor_copy(scrV[:], b2t[:])
            nc.vector.tensor_copy(scrV[:], bpwt[0][:])

            # ---- x0 output tiles (pad 6) ----
            # Block 1 is allocated flat: its storage doubles as the DMA
            # staging area for xb block 1 before stage-1 runs.
            x0p0_t = x0pool.tile([128, 140, 140], bf16, tag="x0p0")
            x0p1_flat = x0pool.tile([128, 19600], bf16, tag="x0p1")
            x0pt = [x0p0_t[:],
                    x0p1_flat[:].rearrange("p (r c) -> p r c", r=140)]

            # ---- padded input tiles ----
            # xb arrives via one contiguous DMA per block (128 fat
            # descriptors); DVE spreads it into the guard-padded layout.
            # A direct strided DMA would need 16K 256-byte descriptors.
            xpt = []
            with tc.tile_pool(name="xstage", bufs=1) as xstage:
                stg = [None, None]
                xs_tile = xstage.tile([128, 128, 128], bf16, tag="xs")
                stg[0] = xs_tile[:]
                nc.gpsimd.dma_start(out=stg[0], in_=xb_ap[0])
                stg[1] = x0p1_flat[:, 0:16384].rearrange(
                    "p (r c) -> p r c", r=128)
                nc.gpsimd.dma_start(out=stg[1], in_=xb_ap[1])
                for blk in range(2):
                    t = xpool.tile([128, 130, 130], bf16, tag=f"xp{blk}")
                    nc.vector.memset(t[:, 0:1, :], 0.0)
                    nc.vector.memset(t[:, 129:130, :], 0.0)
                    nc.vector.memset(t[:, 1:129, 0:1], 0.0)
                    nc.vector.memset(t[:, 1:129, 129:130], 0.0)
                    nc.vector.tensor_copy(t[:, 1:129, 1:129], stg[blk])
                    xpt.append(t)
            tc.no_sync_barrier()

            mult, add = mybir.AluOpType.mult, mybir.AluOpType.add

            def emit_chain(eng, taps, src_of, ktile, bias_ap, out_ap, acc):
                """Depthwise conv as a per-partition-scalar MAC chain:
                acc = x_0*k_0 + bias; acc += x_t*k_t; last writes out_ap."""
                n = len(taps)
                for t in range(n):
                    src = src_of(t)
                    kap = ktile[:, t: t + 1]
                    if t == 0:
                        eng.tensor_scalar(
                            out=acc[:], in0=src, scalar1=kap,
                            scalar2=bias_ap, op0=mult, op1=add)
                    else:
                        eng.scalar_tensor_tensor(
                            out=(out_ap if t == n - 1 else acc[:]),
                            in0=src, scalar=kap, in1=acc[:],
                            op0=mult, op1=add)

            # ---- stage 1 (+ early offloaded branch chains) ----
            # blk0/blk1 interleaved so branch windows (which need both
            # blocks) unblock early. Branch windows assigned to DVE/GpSimd
            # are emitted as soon as their stage-1 inputs exist, into
            # persistent xcoff tiles consumed later by the PW loop.
            offbr = {w: BR_PATTERN[w % len(BR_PATTERN)] for w in range(32)
                     if BR_PATTERN[w % len(BR_PATTERN)] != "p"}
            xcoff = {}
            with (
                tc.tile_pool(name="accs", bufs=2) as accpool,
                tc.tile_pool(name="xcoffp", bufs=1) as xcoffpool,
            ):
                for blk in range(2):
                    xt = x0pt[blk]
                    nc.scalar.memzero(xt[:, 0:6, :])
                    nc.scalar.memzero(xt[:, 134:140, :])
                    nc.scalar.memzero(xt[:, 6:134, 0:6])
                    nc.scalar.memzero(xt[:, 6:134, 134:140])

                def emit_branch_chain(bw, beng):
                    e = nc.vector if beng == "v" else nc.gpsimd
                    rb = bw * 4
                    xo1 = xcoffpool.tile([128, 4, 128], bf16,
                                         tag=f"xco1_{bw}")
                    acc1 = accpool.tile([128, 4, 128], bf16,
                                        tag=f"bracc1_{beng}")
                    emit_chain(
                        e, T25,
                        lambda t: x0pt[1][
                            :, rb + 3 * T25[t][0]: rb + 3 * T25[t][0] + 4,
                            3 * T25[t][1]: 3 * T25[t][1] + 128],
                        k2t, b2t[:], xo1[:], acc1)
                    xo0 = xcoffpool.tile([128, 4, 128], bf16,
                                         tag=f"xco0_{bw}")
                    acc0 = accpool.tile([128, 4, 128], bf16,
                                        tag=f"bracc0_{beng}")
                    emit_chain(
                        e, T9,
                        lambda t: x0pt[0][
                            :, rb + 3 * T9[t][0] + 3: rb + 3 * T9[t][0] + 7,
                            3 * T9[t][1] + 3: 3 * T9[t][1] + 131],
                        k1t, b1t[:], xo0[:], acc0)
                    xcoff[bw] = (xo0, xo1)

                for win in range(32):
                    for blk in range(2):
                        r0 = win * 4
                        eng = S1_PATTERN[(win * 2 + blk) % len(S1_PATTERN)]
                        x0win = x0pt[blk][:, 6 + r0: 6 + r0 + 4, 6:134]
                        if eng == "p":
                            ps = ps1pool.tile([128, 4, 128], f32, tag="s1")
                            for t, (ty, tx) in enumerate(T9):
                                nc.tensor.matmul(
                                    ps[:],
                                    lhsT=d0t[blk][:, t * 128:(t + 1) * 128],
                                    rhs=xpt[blk][:, r0 + ty: r0 + ty + 4,
                                                 tx: tx + 128],
                                    start=(t == 0),
                                    stop=(t == 8),
                                )
                            nc.scalar.activation(
                                out=x0win,
                                in_=ps[:],
                                func=mybir.ActivationFunctionType.Identity,
                                bias=b0t[blk][:],
                                scale=1.0,
                            )
                        else:
                            e = nc.vector if eng == "v" else nc.gpsimd
                            acc = accpool.tile([128, 4, 128], bf16,
                                               tag=f"s1acc_{eng}")
                            emit_chain(
                                e, T9,
                                lambda t: xpt[blk][
                                    :, r0 + T9[t][0]: r0 + T9[t][0] + 4,
                                    T9[t][1]: T9[t][1] + 128],
                                k0t[blk], b0t[blk][:], x0win, acc)
                    for bw, beng in sorted(offbr.items()):
                        if bw not in xcoff and bw + 2 <= win:
                            emit_branch_chain(bw, beng)
                for bw, beng in sorted(offbr.items()):
                    if bw not in xcoff:
                        emit_branch_chain(bw, beng)

            # ---- branches + pointwise, per 4-row window ----
            from concourse.tile_rust import add_dep_helper
            outdma = {0: {}, 1: {}}  # ob -> win -> out-DMA instruction
            with tc.tile_pool(name="bracc", bufs=2) as brpool:
              for win in range(32):
                r0 = win * 4
                beng = BR_PATTERN[win % len(BR_PATTERN)]
                if beng == "p":
                    xc1 = xcpool.tile([128, 4, 128], bf16, tag="xc1")
                    xc0 = xcpool.tile([128, 4, 128], bf16, tag="xc0")
                    # chunk1: x2 (ch 128..191) + x3 (192..255), 5x5 dil3 pad 6
                    ps1 = psbrpool.tile([128, 4, 128], f32, tag="br1")
                    for t, (ty, tx) in enumerate(T25):
                        nc.tensor.matmul(
                            ps1[:],
                            lhsT=d2t[:, t * 128:(t + 1) * 128],
                            rhs=x0pt[1][:, r0 + 3 * ty: r0 + 3 * ty + 4,
                                        3 * tx: 3 * tx + 128],
                            start=(t == 0),
                            stop=(t == 24),
                        )
                    nc.scalar.activation(
                        out=xc1[:], in_=ps1[:],
                        func=mybir.ActivationFunctionType.Identity,
                        bias=b2t[:], scale=1.0,
                    )

                    # chunk0: x0[0:64] pass + x1 (3x3 dil3 pad 3, ch 64..127)
                    ps0 = psbrpool.tile([128, 4, 128], f32, tag="br0")
                    for t, (ty, tx) in enumerate(T9):
                        nc.tensor.matmul(
                            ps0[:],
                            lhsT=d1t[:, t * 128:(t + 1) * 128],
                            rhs=x0pt[0][:, r0 + 3 * ty + 3: r0 + 3 * ty + 7,
                                        3 * tx + 3: 3 * tx + 131],
                            start=(t == 0),
                            stop=(t == 8),
                        )
                    nc.scalar.activation(
                        out=xc0[64:128], in_=ps0[64:128],
                        func=mybir.ActivationFunctionType.Identity,
                        bias=b1t[64:128], scale=1.0,
                    )
                else:
                    xc0, xc1 = xcoff[win]
                nc.scalar.copy(
                    xc0[0:64], x0pt[0][0:64, 6 + r0: 6 + r0 + 4, 6:134]
                )

                # pointwise GEMM; bias folds into the PSUM->SBUF copy
                for ob in range(2):
                    pw = pspwpool.tile([128, 4, 128], f32, tag="pw")
                    nc.tensor.matmul(
                        pw[:], lhsT=wpwt[0][:, ob * 128:(ob + 1) * 128],
                        rhs=xc0[:], start=True, stop=False,
                    )
                    nc.tensor.matmul(
                        pw[:], lhsT=wpwt[1][:, ob * 128:(ob + 1) * 128],
                        rhs=xc1[:], start=False, stop=True,
                    )
                    ys = ystage.tile([128, 4, 128], f32, tag=f"ys{ob}")
                    # The ys slot (bufs=2) is reused every 2 windows; its
                    # WAR dep on the out-DMA would be a 3rd sync wait on the
                    # copy. A scratch dummy absorbs that wait instead.
                    if win >= 2:
                        dum = nc.scalar.copy(scrA[:], scrA[:])
                        add_dep_helper(dum.ins, outdma[ob][win - 2].ins,
                                       sync=True, reason="ys WAR collector")
                    cp = nc.scalar.activation(
                        out=ys[:], in_=pw[:],
                        func=mybir.ActivationFunctionType.Identity,
                        bias=bpwt[ob][:], scale=1.0,
                    )
                    if win >= 2:
                        add_dep_helper(cp.ins, dum.ins, sync=False,
                                       reason="keep collector before copy")
                    outdma[ob][win] = nc.sync.dma_start(
                        out=y_ap[ob * 128:(ob + 1) * 128, r0: r0 + 4, :],
                        in_=ys[:],
                    )
    return nc


def _prep_aux(w0, b0, w1, b1, w2, b2, w_pw, b_pw, bf16):
    d0 = np.zeros((2, 128, 9 * 128), dtype=bf16)
    for blk in range(2):
        for t, (ty, tx) in enumerate(T9):
            np.fill_diagonal(
                d0[blk, :, t * 128:(t + 1) * 128],
                w0[blk * 128:(blk + 1) * 128, 0, ty, tx].astype(bf16),
            )
    d1 = np.zeros((128, 9 * 128), dtype=bf16)
    for t, (ty, tx) in enumerate(T9):
        vals = np.zeros(128, np.float32)
        vals[64:128] = w1[:, 0, ty, tx]
        np.fill_diagonal(d1[:, t * 128:(t + 1) * 128], vals.astype(bf16))
    d2 = np.zeros((128, 25 * 128), dtype=bf16)
    for t, (ty, tx) in enumerate(T25):
        vals = np.concatenate([w2[:, 0, ty, tx], w2[:, 0, ty, tx]])
        np.fill_diagonal(d2[:, t * 128:(t + 1) * 128], vals.astype(bf16))
    wpw = np.zeros((2, 128, 256), dtype=bf16)
    for k in range(2):
        wpw[k] = np.ascontiguousarray(
            w_pw[:, k * 128:(k + 1) * 128].T
        ).astype(bf16)
    bpw = b_pw.reshape(2, 128, 1).astype(np.float32)
    b0v = b0.reshape(2, 128, 1).astype(np.float32)
    b1v = np.zeros((128, 1), np.float32)
    b1v[64:128, 0] = b1
    b2v = np.concatenate([b2, b2]).reshape(128, 1).astype(np.float32)
    # per-partition tap scalars for the DVE/GpSimd MAC-chain paths
    k0sv = np.zeros((2, 128, 9), np.float32)
    for blk in range(2):
        for t, (ty, tx) in enumerate(T9):
            k0sv[blk, :, t] = w0[blk * 128:(blk + 1) * 128, 0, ty, tx]
    k1sv = np.zeros((128, 9), np.float32)
    for t, (ty, tx) in enumerate(T9):
        k1sv[64:128, t] = w1[:, 0, ty, tx]
    k2sv = np.zeros((128, 25), np.float32)
    for t, (ty, tx) in enumerate(T25):
        k2sv[0:64, t] = w2[:, 0, ty, tx]
        k2sv[64:128, t] = w2[:, 0, ty, tx]
    return dict(d0=d0, d1=d1, d2=d2, wpw=wpw, bpw=bpw, b0v=b0v, b1v=b1v,
                b2v=b2v, k0s=k0sv, k1s=k1sv, k2s=k2sv)


def kernel(x, w0, b0, w1, b1, w2, b2, w_pw, b_pw):
    import concourse.mybir as mybir
    from concourse.bass_utils import run_bass_kernel_spmd

    bf16 = mybir.dt.np(mybir.dt.bfloat16)

    if "nc" not in _CACHE:
        nc = _build_nc()
        _split_excess_waits(nc, mybir)
        _CACHE["nc"] = nc
    nc = _CACHE["nc"]

    x = np.asarray(x, np.float32)
    aux = _prep_aux(
        np.asarray(w0, np.float32), np.asarray(b0, np.float32),
        np.asarray(w1, np.float32), np.asarray(b1, np.float32),
        np.asarray(w2, np.float32), np.asarray(b2, np.float32),
        np.asarray(w_pw, np.float32), np.asarray(b_pw, np.float32),
        bf16,
    )
    in_maps = [
        {"xb": np.ascontiguousarray(x[i].reshape(2, 128, H, W)).astype(bf16),
         **aux}
        for i in range(B)
    ]
    res = run_bass_kernel_spmd(nc, in_maps, core_ids=list(range(B)))
    _CACHE["last_result"] = res
    return np.stack([res.results[i]["y"] for i in range(B)]).astype(np.float32)

